# revision 13
# baseline (speedup 1.0000x reference)
"""Trainium2 Bass kernel for ClusterAssignment (vq_codebook, ALPHA=1), V4.

q[n,k] = num[n,k] / sum_k num[n,k],   num = 1/(1 + |z_n - c_k|^2)

V4 = V3 data layout (fp8 z, bf16 q, zsq/csq injected via matmul) with the
elementwise work rebalanced across engines per the documented cost models:
  - the big PSUM->SBUF reciprocal moves from DVE (1x fp32 rate) to the
    otherwise-idle scalar engine's Reciprocal spline (HW-measured 1.2e-5
    max rel err on our input range; bf16 writeback adds only bf16 noise)
  - the small per-row-sum reciprocal also runs on ACT
  - DVE keeps the halving add + segmented reduce (its cheap ops)
  - the normalize multiply q = num * sinv is split between Pool (gpsimd)
    and DVE, ratio tunable via V4_POOL_GROUPS
  - z is loaded in one 1 MiB DMA and q stored in one 1 MiB DMA per pass
    (256 KiB chunks only reach ~65% of DMA peak; 1 MiB reaches ~80%)
"""

import os
import sys

if "/opt/trn_rl_repo" not in sys.path:
    sys.path.insert(0, "/opt/trn_rl_repo")

import ml_dtypes
import numpy as np

import concourse.bacc as bacc
import concourse.tile as tile
from concourse import mybir
from concourse.bass_interp import get_hw_module
from concourse.bass_utils import run_bass_kernel_spmd

N, K, D = 65536, 64, 128
NCORES = 8
NS = N // NCORES  # 8192 rows per core
NSUPER = int(os.environ.get("V4_NSUPER", "4"))
SUP_N = NS // NSUPER
TPS = SUP_N // 128  # tiles (of 128 rows) per superstep
TPB = 8
NB = 2 * TPB + 2
NBANKS = NS // (TPB * 128)  # 8
SBANKS = TPS // TPB  # psum banks per superstep

WARMUP_MM = int(os.environ.get("V4_WARMUP_MM", "10"))
UNROLL = int(os.environ.get("V4_UNROLL", "8"))
POOL_GROUPS = int(os.environ.get("V4_POOL_GROUPS", "14"))  # of TPS=16
Z_CHUNKS = int(os.environ.get("V4_Z_CHUNKS", "1"))
Q_CHUNKS = int(os.environ.get("V4_Q_CHUNKS", "4"))
SRECIP = os.environ.get("V4_SRECIP", "act")  # act | dve
NOHALVE = int(os.environ.get("V4_NOHALVE", "0"))
# queue for each q-store chunk, cycled: sync(SP) / scalar(ACT) / gpsimd(Pool)
QDMA = os.environ.get("V4_QDMA", "sync,gpsimd").split(",")
# timing-only ablations (break correctness; used by timeonly.py)
NO_ZLOAD = int(os.environ.get("V4_NO_ZLOAD", "0"))
NO_QSTORE = int(os.environ.get("V4_NO_QSTORE", "0"))

FP8 = mybir.dt.float8e4
FP8_NP = mybir.dt.np(FP8)

_CACHE = {}


def _act_recip(nc, out, in_, accum_out=None):
    """Emit InstActivation(Reciprocal) directly; bass's activation() refuses
    it for softmax-grade accuracy reasons that don't apply at our 2e-2
    tolerance (HW-measured: 1.2e-5 max rel err over [0.05, 500])."""
    eng = nc.scalar
    inputs = [eng.lower_ap(in_)]
    for arg in (0.0, 1.0, 0.0):  # bias, scale, alpha
        inputs.append(mybir.ImmediateValue(dtype=mybir.dt.float32, value=arg))
    outputs = [eng.lower_ap(out)]
    if accum_out is not None:
        outputs.append(eng.lower_ap(accum_out))
    return eng.add_instruction(
        mybir.InstActivation(
            name=nc.get_next_instruction_name(),
            func=mybir.ActivationFunctionType.Reciprocal,
            ins=inputs,
            outs=outputs,
        )
    )


def _build_nc(iters=1):
    f32 = mybir.dt.float32
    bf16 = mybir.dt.bfloat16
    nc = bacc.Bacc(
        "TRN2",
        target_bir_lowering=False,
        debug=False,
        enable_asserts=False,
        num_devices=NCORES,
    )
    zT = nc.dram_tensor("zT", [D, NS], FP8, kind="ExternalInput").ap()
    cTm2 = nc.dram_tensor("cTm2", [D, K], bf16, kind="ExternalInput").ap()
    blhs = nc.dram_tensor("blhs", [NB, NBANKS * 128], bf16, kind="ExternalInput").ap()
    brhs = nc.dram_tensor("brhs", [NB, TPB * K], bf16, kind="ExternalInput").ap()
    q = nc.dram_tensor("q", [NS, K], bf16, kind="ExternalOutput").ap()

    # row n = s*2048 + p*16 + u; per-pass store view: [p, s, (u k)]
    q_pass = q.rearrange("(s p u) k -> p s (u k)", p=128, u=TPS)

    with tile.TileContext(nc) as tc:
        with (
            tc.tile_pool(name="const", bufs=1) as const_pool,
            tc.tile_pool(name="zin", bufs=3) as zin_pool,
            tc.tile_pool(name="num", bufs=6) as num_pool,
            tc.tile_pool(name="half", bufs=6) as half_pool,
            tc.tile_pool(name="qst", bufs=2) as qst_pool,
            tc.tile_pool(name="small", bufs=16) as small_pool,
            tc.tile_pool(name="psum", bufs=4, space="PSUM") as psum_pool,
        ):
            # zeros for PE warmup: memset early on DVE (idle at t=0)
            warm = const_pool.tile([128, 128], bf16)
            if WARMUP_MM:
                nc.vector.memset(warm[:], 0.0)

            # consts spread across queues so they land in parallel
            c_sb = const_pool.tile([D, K], bf16)
            nc.scalar.dma_start(c_sb[:], cTm2[:])
            blhs_sb = const_pool.tile([NB, NBANKS * 128], bf16)
            nc.gpsimd.dma_start(blhs_sb[:], blhs[:])
            brhs_sb = const_pool.tile([NB, TPB * K], bf16)
            nc.scalar.dma_start(brhs_sb[:], brhs[:])

            # dummy recip so the ACT Reciprocal table-set load (~2.7us)
            # happens at program start, outside the steady-state loop
            dumm = const_pool.tile([128, 1], f32)
            nc.vector.memset(dumm[:], 1.0)
            dummo = const_pool.tile([128, 1], f32)
            _act_recip(nc, dummo[:], dumm[:])

            def z_load(first=False):
                zt = zin_pool.tile([D, NS], FP8, tag="zt")
                if NO_ZLOAD:
                    nc.vector.memset(zt[:, :4], 0.0)
                    return zt
                nchunk = 4 if first else Z_CHUNKS
                step = NS // nchunk
                for c in range(nchunk):
                    nc.sync.dma_start(
                        zt[:, c * step : (c + 1) * step],
                        zT[:, c * step : (c + 1) * step],
                    )
                return zt

            def super_head(s, zt, first=False):
                """PE matmuls for super s, then the big ACT reciprocal.
                Emitted ahead of the previous super's tail so ACT's FIFO
                stream never stalls waiting for the DVE reduce."""
                ps = psum_pool.tile([128, TPS * K], f32, tag="ps")
                if first and WARMUP_MM:
                    for w in range(WARMUP_MM):
                        nc.tensor.matmul(
                            ps[:, :128],
                            warm[:],
                            warm[:],
                            start=True,
                            stop=True,
                            skip_group_check=True,
                        )
                for h in range(SBANKS):
                    b = SBANKS * s + h
                    for t in range(TPB):
                        u = h * TPB + t
                        nc.tensor.matmul(
                            ps[:, u * K : (u + 1) * K],
                            zt[:, (s * TPS + u) * 128 : (s * TPS + u + 1) * 128],
                            c_sb[:],
                            start=(t == 0),
                            stop=False,
                        )
                    nc.tensor.matmul(
                        ps[:, h * TPB * K : (h + 1) * TPB * K],
                        blhs_sb[:, b * 128 : (b + 1) * 128],
                        brhs_sb[:],
                        start=False,
                        stop=True,
                    )

                # big reciprocal on ACT: PSUM f32 -> SBUF bf16
                num = num_pool.tile([128, TPS * K], bf16, tag="num")
                _act_recip(nc, num[:], ps[:])
                return num

            def super_tail(s, num, qstage):
                srow = small_pool.tile([128, TPS], f32, tag="s")
                if NOHALVE:
                    nc.vector.reduce_sum(
                        out=srow[:],
                        in_=num[:].rearrange("p (u k) -> p u k", k=K),
                        axis=mybir.AxisListType.X,
                    )
                else:
                    # halving add then 32-wide segmented reduce, both on DVE
                    nh = half_pool.tile([128, TPS * K // 2], bf16, tag="nh")
                    nv = num[:].rearrange("p (u k) -> p u k", k=K)
                    nc.vector.tensor_add(
                        nh[:].rearrange("p (u k) -> p u k", k=K // 2),
                        nv[:, :, : K // 2],
                        nv[:, :, K // 2 :],
                    )
                    nc.vector.reduce_sum(
                        out=srow[:],
                        in_=nh[:].rearrange("p (u k) -> p u k", k=K // 2),
                        axis=mybir.AxisListType.X,
                    )
                sinv = small_pool.tile([128, TPS], f32, tag="sinv")
                if SRECIP == "act":
                    _act_recip(nc, sinv[:], srow[:])
                else:
                    nc.vector.reciprocal(out=sinv[:], in_=srow[:])

                # normalize: q = num * sinv, split Pool / DVE
                qs = qstage[:, s * TPS * K : (s + 1) * TPS * K]
                g = POOL_GROUPS
                if g > 0:
                    nc.gpsimd.tensor_mul(
                        qs[:, : g * K].rearrange("p (u k) -> p u k", k=K),
                        num[:, : g * K].rearrange("p (u k) -> p u k", k=K),
                        sinv[:, :g].broadcast_to([128, g, K]),
                    )
                if g < TPS:
                    nc.vector.tensor_mul(
                        qs[:, g * K :].rearrange("p (u k) -> p u k", k=K),
                        num[:, g * K :].rearrange("p (u k) -> p u k", k=K),
                        sinv[:, g:].broadcast_to([128, TPS - g, K]),
                    )

            def body(first=False):
                zt = z_load(first=first)
                qstage = qst_pool.tile([128, NSUPER * TPS * K], bf16, tag="qst")
                nums = {}
                for s in range(NSUPER):
                    nums[s] = super_head(s, zt, first=(first and s == 0))
                    if s >= 1:
                        super_tail(s - 1, nums[s - 1], qstage)
                super_tail(NSUPER - 1, nums[NSUPER - 1], qstage)
                if NO_QSTORE:
                    return
                qsv = qstage[:].rearrange("p (s x) -> p s x", x=TPS * K)
                step = NSUPER // Q_CHUNKS
                for c in range(Q_CHUNKS):
                    eng = getattr(nc, QDMA[c % len(QDMA)])
                    eng.dma_start(
                        q_pass[:, c * step : (c + 1) * step, :],
                        qsv[:, c * step : (c + 1) * step, :],
                    )

            if iters == 1:
                body(first=True)
            else:
                if WARMUP_MM:
                    ps_warm = psum_pool.tile([128, TPS * K], f32, tag="ps")
                    for w in range(WARMUP_MM):
                        nc.tensor.matmul(
                            ps_warm[:, :128],
                            warm[:],
                            warm[:],
                            start=True,
                            stop=True,
                            skip_group_check=True,
                        )
                u = UNROLL if iters % UNROLL == 0 and iters > UNROLL else 1
                with tc.For_i(0, iters // u, 1, staggered_reset=True):
                    for _ in range(u):
                        body(first=False)

    nc.compile()
    nc.m = get_hw_module(nc.m)
    return nc


def _get_nc():
    if "nc" not in _CACHE:
        _CACHE["nc"] = _build_nc()
    return _CACHE["nc"]


def _hilo(x):
    hi = x.astype(ml_dtypes.bfloat16)
    lo = (x - hi.astype(np.float64)).astype(ml_dtypes.bfloat16)
    return hi, lo


def _host_prep(z, centroids):
    z = np.asarray(z, dtype=np.float32)
    c = np.asarray(centroids, dtype=np.float32)

    cm2_bf = ((-2.0 * c.T).astype(ml_dtypes.bfloat16)).astype(np.float64)  # [D,K]
    c_eff = -0.5 * cm2_bf
    csq1 = 1.0 + (c_eff**2).sum(axis=0)
    csq1_hi, csq1_lo = _hilo(csq1)

    brhs = np.zeros((NB, TPB * K), dtype=ml_dtypes.bfloat16)
    for t in range(TPB):
        brhs[t, t * K : (t + 1) * K] = 1.0
        brhs[TPB + t, t * K : (t + 1) * K] = 1.0
    brhs[2 * TPB, :] = np.tile(csq1_hi, TPB)
    brhs[2 * TPB + 1, :] = np.tile(csq1_lo, TPB)

    in_maps = []
    for i in range(NCORES):
        zs = z[i * NS : (i + 1) * NS]
        z_perm = (
            zs.reshape(NSUPER, 128, TPS, D).transpose(0, 2, 1, 3).reshape(NS, D)
        )
        zT8 = np.ascontiguousarray(z_perm.T).astype(FP8_NP)

        z_eff = zT8.astype(np.float64).T
        zsq_perm = (z_eff**2).sum(axis=1)
        zsq_hi, zsq_lo = _hilo(zsq_perm)
        blhs = np.empty((NB, NBANKS * 128), dtype=ml_dtypes.bfloat16)
        hi = zsq_hi.reshape(NSUPER, TPS, 128).reshape(NSUPER, 2, TPB, 128)
        lo = zsq_lo.reshape(NSUPER, TPS, 128).reshape(NSUPER, 2, TPB, 128)
        blhs[:TPB] = hi.transpose(2, 0, 1, 3).reshape(TPB, -1)
        blhs[TPB : 2 * TPB] = lo.transpose(2, 0, 1, 3).reshape(TPB, -1)
        blhs[2 * TPB :] = 1.0
        in_maps.append(
            {
                "zT": zT8,
                "cTm2": cm2_bf.astype(ml_dtypes.bfloat16),
                "blhs": blhs,
                "brhs": brhs,
            }
        )
    return in_maps


def kernel(z, centroids):
    nc = _get_nc()
    in_maps = _host_prep(z, centroids)
    res = run_bass_kernel_spmd(nc, in_maps, list(range(NCORES)))
    out = np.concatenate(
        [np.asarray(res.results[i]["q"]) for i in range(NCORES)], axis=0
    )
    return out.astype(np.float32)


# revision 14
# speedup vs baseline: 1.1004x; 1.1004x over previous
"""Trainium2 Bass kernel for ClusterAssignment (vq_codebook, ALPHA=1), V4.

q[n,k] = num[n,k] / sum_k num[n,k],   num = 1/(1 + |z_n - c_k|^2)

V4 = V3 data layout (fp8 z, bf16 q, zsq/csq injected via matmul) with the
elementwise work rebalanced across engines per the documented cost models:
  - the big PSUM->SBUF reciprocal moves from DVE (1x fp32 rate) to the
    otherwise-idle scalar engine's Reciprocal spline (HW-measured 1.2e-5
    max rel err on our input range; bf16 writeback adds only bf16 noise)
  - the small per-row-sum reciprocal also runs on ACT
  - DVE keeps the halving add + segmented reduce (its cheap ops)
  - the normalize multiply q = num * sinv is split between Pool (gpsimd)
    and DVE, ratio tunable via V4_POOL_GROUPS
  - z is loaded in one 1 MiB DMA and q stored in one 1 MiB DMA per pass
    (256 KiB chunks only reach ~65% of DMA peak; 1 MiB reaches ~80%)
"""

import os
import sys

if "/opt/trn_rl_repo" not in sys.path:
    sys.path.insert(0, "/opt/trn_rl_repo")

import ml_dtypes
import numpy as np

import concourse.bacc as bacc
import concourse.tile as tile
from concourse import mybir
from concourse.bass_interp import get_hw_module
from concourse.bass_utils import run_bass_kernel_spmd

N, K, D = 65536, 64, 128
NCORES = 8
NS = N // NCORES  # 8192 rows per core
NSUPER = int(os.environ.get("V4_NSUPER", "4"))
SUP_N = NS // NSUPER
TPS = SUP_N // 128  # tiles (of 128 rows) per superstep
TPB = 8
NB = 2 * TPB + 2
NBANKS = NS // (TPB * 128)  # 8
SBANKS = TPS // TPB  # psum banks per superstep

WARMUP_MM = int(os.environ.get("V4_WARMUP_MM", "10"))
UNROLL = int(os.environ.get("V4_UNROLL", "8"))
POOL_GROUPS = int(os.environ.get("V4_POOL_GROUPS", "14"))  # of TPS=16
Z_CHUNKS = int(os.environ.get("V4_Z_CHUNKS", "1"))
Q_CHUNKS = int(os.environ.get("V4_Q_CHUNKS", "4"))
SRECIP = os.environ.get("V4_SRECIP", "act")  # act | dve
NOHALVE = int(os.environ.get("V4_NOHALVE", "0"))
# queue for each q-store chunk, cycled: sync(SP) / scalar(ACT) / gpsimd(Pool)
QDMA = os.environ.get("V4_QDMA", "sync,gpsimd").split(",")
# timing-only ablations (break correctness; used by timeonly.py)
NO_ZLOAD = int(os.environ.get("V4_NO_ZLOAD", "0"))
NO_QSTORE = int(os.environ.get("V4_NO_QSTORE", "0"))

FP8 = mybir.dt.float8e4
FP8_NP = mybir.dt.np(FP8)

_CACHE = {}


def _act_recip(nc, out, in_, accum_out=None):
    """Emit InstActivation(Reciprocal) directly; bass's activation() refuses
    it for softmax-grade accuracy reasons that don't apply at our 2e-2
    tolerance (HW-measured: 1.2e-5 max rel err over [0.05, 500])."""
    eng = nc.scalar
    inputs = [eng.lower_ap(in_)]
    for arg in (0.0, 1.0, 0.0):  # bias, scale, alpha
        inputs.append(mybir.ImmediateValue(dtype=mybir.dt.float32, value=arg))
    outputs = [eng.lower_ap(out)]
    if accum_out is not None:
        outputs.append(eng.lower_ap(accum_out))
    return eng.add_instruction(
        mybir.InstActivation(
            name=nc.get_next_instruction_name(),
            func=mybir.ActivationFunctionType.Reciprocal,
            ins=inputs,
            outs=outputs,
        )
    )


def _build_nc(iters=1):
    f32 = mybir.dt.float32
    bf16 = mybir.dt.bfloat16
    nc = bacc.Bacc(
        "TRN2",
        target_bir_lowering=False,
        debug=False,
        enable_asserts=False,
        num_devices=NCORES,
    )
    zT = nc.dram_tensor("zT", [D, NS], FP8, kind="ExternalInput").ap()
    cTm2 = nc.dram_tensor("cTm2", [D, K], bf16, kind="ExternalInput").ap()
    blhs = nc.dram_tensor("blhs", [NB, NBANKS * 128], bf16, kind="ExternalInput").ap()
    brhs = nc.dram_tensor("brhs", [NB, TPB * K], bf16, kind="ExternalInput").ap()
    q = nc.dram_tensor("q", [NS, K], bf16, kind="ExternalOutput").ap()

    # row n = s*2048 + p*16 + u; per-pass store view: [p, s, (u k)]
    q_pass = q.rearrange("(s p u) k -> p s (u k)", p=128, u=TPS)

    with tile.TileContext(nc) as tc:
        with (
            tc.tile_pool(name="const", bufs=1) as const_pool,
            tc.tile_pool(name="zin", bufs=3) as zin_pool,
            tc.tile_pool(name="num", bufs=6) as num_pool,
            tc.tile_pool(name="half", bufs=6) as half_pool,
            tc.tile_pool(name="qst", bufs=2) as qst_pool,
            tc.tile_pool(name="small", bufs=16) as small_pool,
            tc.tile_pool(
                name="psum", bufs=min(4, 8 // SBANKS), space="PSUM"
            ) as psum_pool,
        ):
            # zeros for PE warmup: memset early on DVE (idle at t=0)
            warm = const_pool.tile([128, 128], bf16)
            if WARMUP_MM:
                nc.vector.memset(warm[:], 0.0)

            # consts spread across queues so they land in parallel
            c_sb = const_pool.tile([D, K], bf16)
            nc.scalar.dma_start(c_sb[:], cTm2[:])
            blhs_sb = const_pool.tile([NB, NBANKS * 128], bf16)
            nc.gpsimd.dma_start(blhs_sb[:], blhs[:])
            brhs_sb = const_pool.tile([NB, TPB * K], bf16)
            nc.scalar.dma_start(brhs_sb[:], brhs[:])

            # dummy recip so the ACT Reciprocal table-set load (~2.7us)
            # happens at program start, outside the steady-state loop
            dumm = const_pool.tile([128, 1], f32)
            nc.vector.memset(dumm[:], 1.0)
            dummo = const_pool.tile([128, 1], f32)
            _act_recip(nc, dummo[:], dumm[:])

            def z_load(first=False):
                zt = zin_pool.tile([D, NS], FP8, tag="zt")
                if NO_ZLOAD:
                    nc.vector.memset(zt[:, :4], 0.0)
                    return zt
                nchunk = 4 if first else Z_CHUNKS
                step = NS // nchunk
                for c in range(nchunk):
                    nc.sync.dma_start(
                        zt[:, c * step : (c + 1) * step],
                        zT[:, c * step : (c + 1) * step],
                    )
                return zt

            def super_head(s, zt, first=False):
                """PE matmuls for super s, then the big ACT reciprocal.
                Emitted ahead of the previous super's tail so ACT's FIFO
                stream never stalls waiting for the DVE reduce."""
                ps = psum_pool.tile([128, TPS * K], f32, tag="ps")
                if first and WARMUP_MM:
                    for w in range(WARMUP_MM):
                        nc.tensor.matmul(
                            ps[:, :128],
                            warm[:],
                            warm[:],
                            start=True,
                            stop=True,
                            skip_group_check=True,
                        )
                for h in range(SBANKS):
                    b = SBANKS * s + h
                    for t in range(TPB):
                        u = h * TPB + t
                        nc.tensor.matmul(
                            ps[:, u * K : (u + 1) * K],
                            zt[:, (s * TPS + u) * 128 : (s * TPS + u + 1) * 128],
                            c_sb[:],
                            start=(t == 0),
                            stop=False,
                        )
                    nc.tensor.matmul(
                        ps[:, h * TPB * K : (h + 1) * TPB * K],
                        blhs_sb[:, b * 128 : (b + 1) * 128],
                        brhs_sb[:],
                        start=False,
                        stop=True,
                    )

                # big reciprocal on ACT: PSUM f32 -> SBUF bf16
                num = num_pool.tile([128, TPS * K], bf16, tag="num")
                _act_recip(nc, num[:], ps[:])
                return num

            def super_tail(s, num, qstage):
                srow = small_pool.tile([128, TPS], f32, tag="s")
                if NOHALVE:
                    nc.vector.reduce_sum(
                        out=srow[:],
                        in_=num[:].rearrange("p (u k) -> p u k", k=K),
                        axis=mybir.AxisListType.X,
                    )
                else:
                    # halving add then 32-wide segmented reduce, both on DVE
                    nh = half_pool.tile([128, TPS * K // 2], bf16, tag="nh")
                    nv = num[:].rearrange("p (u k) -> p u k", k=K)
                    nc.vector.tensor_add(
                        nh[:].rearrange("p (u k) -> p u k", k=K // 2),
                        nv[:, :, : K // 2],
                        nv[:, :, K // 2 :],
                    )
                    nc.vector.reduce_sum(
                        out=srow[:],
                        in_=nh[:].rearrange("p (u k) -> p u k", k=K // 2),
                        axis=mybir.AxisListType.X,
                    )
                sinv = small_pool.tile([128, TPS], f32, tag="sinv")
                if SRECIP == "act":
                    _act_recip(nc, sinv[:], srow[:])
                else:
                    nc.vector.reciprocal(out=sinv[:], in_=srow[:])

                # normalize: q = num * sinv, split Pool / DVE
                qs = qstage[:, s * TPS * K : (s + 1) * TPS * K]
                g = POOL_GROUPS
                if g > 0:
                    nc.gpsimd.tensor_mul(
                        qs[:, : g * K].rearrange("p (u k) -> p u k", k=K),
                        num[:, : g * K].rearrange("p (u k) -> p u k", k=K),
                        sinv[:, :g].broadcast_to([128, g, K]),
                    )
                if g < TPS:
                    nc.vector.tensor_mul(
                        qs[:, g * K :].rearrange("p (u k) -> p u k", k=K),
                        num[:, g * K :].rearrange("p (u k) -> p u k", k=K),
                        sinv[:, g:].broadcast_to([128, TPS - g, K]),
                    )

            def body(first=False):
                zt = z_load(first=first)
                qstage = qst_pool.tile([128, NSUPER * TPS * K], bf16, tag="qst")
                nums = {}
                for s in range(NSUPER):
                    nums[s] = super_head(s, zt, first=(first and s == 0))
                    if s >= 1:
                        super_tail(s - 1, nums[s - 1], qstage)
                super_tail(NSUPER - 1, nums[NSUPER - 1], qstage)
                if NO_QSTORE:
                    return
                qsv = qstage[:].rearrange("p (s x) -> p s x", x=TPS * K)
                step = NSUPER // Q_CHUNKS
                for c in range(Q_CHUNKS):
                    eng = getattr(nc, QDMA[c % len(QDMA)])
                    eng.dma_start(
                        q_pass[:, c * step : (c + 1) * step, :],
                        qsv[:, c * step : (c + 1) * step, :],
                    )

            if iters == 1:
                body(first=True)
            else:
                if WARMUP_MM:
                    ps_warm = psum_pool.tile([128, TPS * K], f32, tag="ps")
                    for w in range(WARMUP_MM):
                        nc.tensor.matmul(
                            ps_warm[:, :128],
                            warm[:],
                            warm[:],
                            start=True,
                            stop=True,
                            skip_group_check=True,
                        )
                u = UNROLL if iters % UNROLL == 0 and iters > UNROLL else 1
                with tc.For_i(0, iters // u, 1, staggered_reset=True):
                    for _ in range(u):
                        body(first=False)

    nc.compile()
    nc.m = get_hw_module(nc.m)
    return nc


def _get_nc():
    if "nc" not in _CACHE:
        _CACHE["nc"] = _build_nc()
    return _CACHE["nc"]


def _hilo(x):
    hi = x.astype(ml_dtypes.bfloat16)
    lo = (x - hi.astype(np.float64)).astype(ml_dtypes.bfloat16)
    return hi, lo


def _host_prep(z, centroids):
    z = np.asarray(z, dtype=np.float32)
    c = np.asarray(centroids, dtype=np.float32)

    cm2_bf = ((-2.0 * c.T).astype(ml_dtypes.bfloat16)).astype(np.float64)  # [D,K]
    c_eff = -0.5 * cm2_bf
    csq1 = 1.0 + (c_eff**2).sum(axis=0)
    csq1_hi, csq1_lo = _hilo(csq1)

    brhs = np.zeros((NB, TPB * K), dtype=ml_dtypes.bfloat16)
    for t in range(TPB):
        brhs[t, t * K : (t + 1) * K] = 1.0
        brhs[TPB + t, t * K : (t + 1) * K] = 1.0
    brhs[2 * TPB, :] = np.tile(csq1_hi, TPB)
    brhs[2 * TPB + 1, :] = np.tile(csq1_lo, TPB)

    in_maps = []
    for i in range(NCORES):
        zs = z[i * NS : (i + 1) * NS]
        z_perm = (
            zs.reshape(NSUPER, 128, TPS, D).transpose(0, 2, 1, 3).reshape(NS, D)
        )
        zT8 = np.ascontiguousarray(z_perm.T).astype(FP8_NP)

        z_eff = zT8.astype(np.float64).T
        zsq_perm = (z_eff**2).sum(axis=1)
        zsq_hi, zsq_lo = _hilo(zsq_perm)
        blhs = np.empty((NB, NBANKS * 128), dtype=ml_dtypes.bfloat16)
        hi = zsq_hi.reshape(NSUPER, TPS, 128).reshape(NSUPER, 2, TPB, 128)
        lo = zsq_lo.reshape(NSUPER, TPS, 128).reshape(NSUPER, 2, TPB, 128)
        blhs[:TPB] = hi.transpose(2, 0, 1, 3).reshape(TPB, -1)
        blhs[TPB : 2 * TPB] = lo.transpose(2, 0, 1, 3).reshape(TPB, -1)
        blhs[2 * TPB :] = 1.0
        in_maps.append(
            {
                "zT": zT8,
                "cTm2": cm2_bf.astype(ml_dtypes.bfloat16),
                "blhs": blhs,
                "brhs": brhs,
            }
        )
    return in_maps


def kernel(z, centroids):
    nc = _get_nc()
    in_maps = _host_prep(z, centroids)
    res = run_bass_kernel_spmd(nc, in_maps, list(range(NCORES)))
    out = np.concatenate(
        [np.asarray(res.results[i]["q"]) for i in range(NCORES)], axis=0
    )
    return out.astype(np.float32)


# revision 15
# speedup vs baseline: 1.1711x; 1.0642x over previous
"""Trainium2 Bass kernel for ClusterAssignment (vq_codebook, ALPHA=1), V4.

q[n,k] = num[n,k] / sum_k num[n,k],   num = 1/(1 + |z_n - c_k|^2)

V4 = V3 data layout (fp8 z, bf16 q, zsq/csq injected via matmul) with the
elementwise work rebalanced across engines per the documented cost models:
  - the big PSUM->SBUF reciprocal moves from DVE (1x fp32 rate) to the
    otherwise-idle scalar engine's Reciprocal spline (HW-measured 1.2e-5
    max rel err on our input range; bf16 writeback adds only bf16 noise)
  - the small per-row-sum reciprocal also runs on ACT
  - DVE keeps the halving add + segmented reduce (its cheap ops)
  - the normalize multiply q = num * sinv is split between Pool (gpsimd)
    and DVE, ratio tunable via V4_POOL_GROUPS
  - z is loaded in one 1 MiB DMA and q stored in one 1 MiB DMA per pass
    (256 KiB chunks only reach ~65% of DMA peak; 1 MiB reaches ~80%)
"""

import os
import sys

if "/opt/trn_rl_repo" not in sys.path:
    sys.path.insert(0, "/opt/trn_rl_repo")

import ml_dtypes
import numpy as np

import concourse.bacc as bacc
import concourse.tile as tile
from concourse import mybir
from concourse.bass_interp import get_hw_module
from concourse.bass_utils import run_bass_kernel_spmd

N, K, D = 65536, 64, 128
NCORES = 8
NS = N // NCORES  # 8192 rows per core
NSUPER = int(os.environ.get("V4_NSUPER", "4"))
SUP_N = NS // NSUPER
TPS = SUP_N // 128  # tiles (of 128 rows) per superstep
TPB = 8
NB = 2 * TPB + 2
NBANKS = NS // (TPB * 128)  # 8
SBANKS = TPS // TPB  # psum banks per superstep

WARMUP_MM = int(os.environ.get("V4_WARMUP_MM", "10"))
UNROLL = int(os.environ.get("V4_UNROLL", "20"))
POOL_GROUPS = int(os.environ.get("V4_POOL_GROUPS", "16"))  # of TPS=16
Z_CHUNKS = int(os.environ.get("V4_Z_CHUNKS", "1"))
Q_CHUNKS = int(os.environ.get("V4_Q_CHUNKS", "2"))
SRECIP = os.environ.get("V4_SRECIP", "act")  # act | dve
NOHALVE = int(os.environ.get("V4_NOHALVE", "1"))
# queue for each q-store chunk, cycled: sync(SP) / scalar(ACT) / gpsimd(Pool)
QDMA = os.environ.get("V4_QDMA", "sync,gpsimd").split(",")
# timing-only ablations (break correctness; used by timeonly.py)
NO_ZLOAD = int(os.environ.get("V4_NO_ZLOAD", "0"))
NO_QSTORE = int(os.environ.get("V4_NO_QSTORE", "0"))

FP8 = mybir.dt.float8e4
FP8_NP = mybir.dt.np(FP8)

_CACHE = {}


def _act_recip(nc, out, in_, accum_out=None):
    """Emit InstActivation(Reciprocal) directly; bass's activation() refuses
    it for softmax-grade accuracy reasons that don't apply at our 2e-2
    tolerance (HW-measured: 1.2e-5 max rel err over [0.05, 500])."""
    eng = nc.scalar
    inputs = [eng.lower_ap(in_)]
    for arg in (0.0, 1.0, 0.0):  # bias, scale, alpha
        inputs.append(mybir.ImmediateValue(dtype=mybir.dt.float32, value=arg))
    outputs = [eng.lower_ap(out)]
    if accum_out is not None:
        outputs.append(eng.lower_ap(accum_out))
    return eng.add_instruction(
        mybir.InstActivation(
            name=nc.get_next_instruction_name(),
            func=mybir.ActivationFunctionType.Reciprocal,
            ins=inputs,
            outs=outputs,
        )
    )


def _build_nc(iters=1):
    f32 = mybir.dt.float32
    bf16 = mybir.dt.bfloat16
    nc = bacc.Bacc(
        "TRN2",
        target_bir_lowering=False,
        debug=False,
        enable_asserts=False,
        num_devices=NCORES,
    )
    zT = nc.dram_tensor("zT", [D, NS], FP8, kind="ExternalInput").ap()
    cTm2 = nc.dram_tensor("cTm2", [D, K], bf16, kind="ExternalInput").ap()
    blhs = nc.dram_tensor("blhs", [NB, NBANKS * 128], bf16, kind="ExternalInput").ap()
    brhs = nc.dram_tensor("brhs", [NB, TPB * K], bf16, kind="ExternalInput").ap()
    q = nc.dram_tensor("q", [NS, K], bf16, kind="ExternalOutput").ap()

    # row n = s*2048 + p*16 + u; per-pass store view: [p, s, (u k)]
    q_pass = q.rearrange("(s p u) k -> p s (u k)", p=128, u=TPS)

    with tile.TileContext(nc) as tc:
        with (
            tc.tile_pool(name="const", bufs=1) as const_pool,
            tc.tile_pool(name="zin", bufs=3) as zin_pool,
            tc.tile_pool(name="num", bufs=6) as num_pool,
            tc.tile_pool(name="half", bufs=6) as half_pool,
            tc.tile_pool(name="qst", bufs=2) as qst_pool,
            tc.tile_pool(name="small", bufs=16) as small_pool,
            tc.tile_pool(
                name="psum", bufs=min(4, 8 // SBANKS), space="PSUM"
            ) as psum_pool,
        ):
            # zeros for PE warmup: memset early on DVE (idle at t=0)
            warm = const_pool.tile([128, 128], bf16)
            if WARMUP_MM:
                nc.vector.memset(warm[:], 0.0)

            # consts spread across queues so they land in parallel
            c_sb = const_pool.tile([D, K], bf16)
            nc.scalar.dma_start(c_sb[:], cTm2[:])
            blhs_sb = const_pool.tile([NB, NBANKS * 128], bf16)
            nc.gpsimd.dma_start(blhs_sb[:], blhs[:])
            brhs_sb = const_pool.tile([NB, TPB * K], bf16)
            nc.scalar.dma_start(brhs_sb[:], brhs[:])

            # dummy recip so the ACT Reciprocal table-set load (~2.7us)
            # happens at program start, outside the steady-state loop
            dumm = const_pool.tile([128, 1], f32)
            nc.vector.memset(dumm[:], 1.0)
            dummo = const_pool.tile([128, 1], f32)
            _act_recip(nc, dummo[:], dumm[:])

            def z_load(first=False):
                zt = zin_pool.tile([D, NS], FP8, tag="zt")
                if NO_ZLOAD:
                    nc.vector.memset(zt[:, :4], 0.0)
                    return zt
                nchunk = 4 if first else Z_CHUNKS
                step = NS // nchunk
                for c in range(nchunk):
                    nc.sync.dma_start(
                        zt[:, c * step : (c + 1) * step],
                        zT[:, c * step : (c + 1) * step],
                    )
                return zt

            def super_head(s, zt, first=False):
                """PE matmuls for super s, then the big ACT reciprocal.
                Emitted ahead of the previous super's tail so ACT's FIFO
                stream never stalls waiting for the DVE reduce."""
                ps = psum_pool.tile([128, TPS * K], f32, tag="ps")
                if first and WARMUP_MM:
                    for w in range(WARMUP_MM):
                        nc.tensor.matmul(
                            ps[:, :128],
                            warm[:],
                            warm[:],
                            start=True,
                            stop=True,
                            skip_group_check=True,
                        )
                for h in range(SBANKS):
                    b = SBANKS * s + h
                    for t in range(TPB):
                        u = h * TPB + t
                        nc.tensor.matmul(
                            ps[:, u * K : (u + 1) * K],
                            zt[:, (s * TPS + u) * 128 : (s * TPS + u + 1) * 128],
                            c_sb[:],
                            start=(t == 0),
                            stop=False,
                        )
                    nc.tensor.matmul(
                        ps[:, h * TPB * K : (h + 1) * TPB * K],
                        blhs_sb[:, b * 128 : (b + 1) * 128],
                        brhs_sb[:],
                        start=False,
                        stop=True,
                    )

                # big reciprocal on ACT: PSUM f32 -> SBUF bf16
                num = num_pool.tile([128, TPS * K], bf16, tag="num")
                _act_recip(nc, num[:], ps[:])
                return num

            def super_tail(s, num, qstage):
                srow = small_pool.tile([128, TPS], f32, tag="s")
                if NOHALVE:
                    nc.vector.reduce_sum(
                        out=srow[:],
                        in_=num[:].rearrange("p (u k) -> p u k", k=K),
                        axis=mybir.AxisListType.X,
                    )
                else:
                    # halving add then 32-wide segmented reduce, both on DVE
                    nh = half_pool.tile([128, TPS * K // 2], bf16, tag="nh")
                    nv = num[:].rearrange("p (u k) -> p u k", k=K)
                    nc.vector.tensor_add(
                        nh[:].rearrange("p (u k) -> p u k", k=K // 2),
                        nv[:, :, : K // 2],
                        nv[:, :, K // 2 :],
                    )
                    nc.vector.reduce_sum(
                        out=srow[:],
                        in_=nh[:].rearrange("p (u k) -> p u k", k=K // 2),
                        axis=mybir.AxisListType.X,
                    )
                sinv = small_pool.tile([128, TPS], f32, tag="sinv")
                if SRECIP == "act":
                    _act_recip(nc, sinv[:], srow[:])
                else:
                    nc.vector.reciprocal(out=sinv[:], in_=srow[:])

                # normalize: q = num * sinv, split Pool / DVE
                qs = qstage[:, s * TPS * K : (s + 1) * TPS * K]
                g = POOL_GROUPS
                if g > 0:
                    nc.gpsimd.tensor_mul(
                        qs[:, : g * K].rearrange("p (u k) -> p u k", k=K),
                        num[:, : g * K].rearrange("p (u k) -> p u k", k=K),
                        sinv[:, :g].broadcast_to([128, g, K]),
                    )
                if g < TPS:
                    nc.vector.tensor_mul(
                        qs[:, g * K :].rearrange("p (u k) -> p u k", k=K),
                        num[:, g * K :].rearrange("p (u k) -> p u k", k=K),
                        sinv[:, g:].broadcast_to([128, TPS - g, K]),
                    )

            def body(first=False):
                zt = z_load(first=first)
                qstage = qst_pool.tile([128, NSUPER * TPS * K], bf16, tag="qst")
                nums = {}
                for s in range(NSUPER):
                    nums[s] = super_head(s, zt, first=(first and s == 0))
                    if s >= 1:
                        super_tail(s - 1, nums[s - 1], qstage)
                super_tail(NSUPER - 1, nums[NSUPER - 1], qstage)
                if NO_QSTORE:
                    return
                qsv = qstage[:].rearrange("p (s x) -> p s x", x=TPS * K)
                step = NSUPER // Q_CHUNKS
                for c in range(Q_CHUNKS):
                    eng = getattr(nc, QDMA[c % len(QDMA)])
                    eng.dma_start(
                        q_pass[:, c * step : (c + 1) * step, :],
                        qsv[:, c * step : (c + 1) * step, :],
                    )

            if iters == 1:
                body(first=True)
            else:
                if WARMUP_MM:
                    ps_warm = psum_pool.tile([128, TPS * K], f32, tag="ps")
                    for w in range(WARMUP_MM):
                        nc.tensor.matmul(
                            ps_warm[:, :128],
                            warm[:],
                            warm[:],
                            start=True,
                            stop=True,
                            skip_group_check=True,
                        )
                u = UNROLL if iters % UNROLL == 0 and iters > UNROLL else 1
                with tc.For_i(0, iters // u, 1, staggered_reset=True):
                    for _ in range(u):
                        body(first=False)

    nc.compile()
    nc.m = get_hw_module(nc.m)
    return nc


def _get_nc():
    if "nc" not in _CACHE:
        _CACHE["nc"] = _build_nc()
    return _CACHE["nc"]


def _hilo(x):
    hi = x.astype(ml_dtypes.bfloat16)
    lo = (x - hi.astype(np.float64)).astype(ml_dtypes.bfloat16)
    return hi, lo


def _host_prep(z, centroids):
    z = np.asarray(z, dtype=np.float32)
    c = np.asarray(centroids, dtype=np.float32)

    cm2_bf = ((-2.0 * c.T).astype(ml_dtypes.bfloat16)).astype(np.float64)  # [D,K]
    c_eff = -0.5 * cm2_bf
    csq1 = 1.0 + (c_eff**2).sum(axis=0)
    csq1_hi, csq1_lo = _hilo(csq1)

    brhs = np.zeros((NB, TPB * K), dtype=ml_dtypes.bfloat16)
    for t in range(TPB):
        brhs[t, t * K : (t + 1) * K] = 1.0
        brhs[TPB + t, t * K : (t + 1) * K] = 1.0
    brhs[2 * TPB, :] = np.tile(csq1_hi, TPB)
    brhs[2 * TPB + 1, :] = np.tile(csq1_lo, TPB)

    in_maps = []
    for i in range(NCORES):
        zs = z[i * NS : (i + 1) * NS]
        z_perm = (
            zs.reshape(NSUPER, 128, TPS, D).transpose(0, 2, 1, 3).reshape(NS, D)
        )
        zT8 = np.ascontiguousarray(z_perm.T).astype(FP8_NP)

        z_eff = zT8.astype(np.float64).T
        zsq_perm = (z_eff**2).sum(axis=1)
        zsq_hi, zsq_lo = _hilo(zsq_perm)
        blhs = np.empty((NB, NBANKS * 128), dtype=ml_dtypes.bfloat16)
        hi = zsq_hi.reshape(NSUPER, TPS, 128).reshape(NSUPER, 2, TPB, 128)
        lo = zsq_lo.reshape(NSUPER, TPS, 128).reshape(NSUPER, 2, TPB, 128)
        blhs[:TPB] = hi.transpose(2, 0, 1, 3).reshape(TPB, -1)
        blhs[TPB : 2 * TPB] = lo.transpose(2, 0, 1, 3).reshape(TPB, -1)
        blhs[2 * TPB :] = 1.0
        in_maps.append(
            {
                "zT": zT8,
                "cTm2": cm2_bf.astype(ml_dtypes.bfloat16),
                "blhs": blhs,
                "brhs": brhs,
            }
        )
    return in_maps


def kernel(z, centroids):
    nc = _get_nc()
    in_maps = _host_prep(z, centroids)
    res = run_bass_kernel_spmd(nc, in_maps, list(range(NCORES)))
    out = np.concatenate(
        [np.asarray(res.results[i]["q"]) for i in range(NCORES)], axis=0
    )
    return out.astype(np.float32)


# revision 16
# speedup vs baseline: 1.2278x; 1.0484x over previous
"""Trainium2 Bass kernel for ClusterAssignment (vq_codebook, ALPHA=1), V5.

q[n,k] = num[n,k] / sum_k num[n,k],   num = 1/(1 + |z_n - c_k|^2)

V5 = V3 data layout (fp8 z in, bf16 q out, zsq/csq injected via matmul)
with the pipeline restructured around what HW experiments showed actually
binds (engine-FIFO stalls and For_i body overheads, not DMA bandwidth):
  - big PSUM->SBUF reciprocal on the scalar engine's Reciprocal spline
    (HW-measured 1.2e-5 max rel err on our range; bass blocks it only for
    softmax-grade uses), small per-row-sum reciprocal also on ACT
  - software-pipelined emission: super s+1's matmuls + big recip are
    emitted BEFORE super s's reduce/srecip/mul, so ACT's strict-FIFO
    stream never stalls waiting on the DVE reduce (this was worth ~2 us)
  - DVE does one segmented reduce per super (NOHALVE); the whole
    normalize multiply runs on Pool (gpsimd) - keeping small ops off
    busy engines' FIFOs beats theoretical per-element balance
  - z loaded in one 1 MiB sync-queue DMA; q stored in two half-pass
    chunks on sync+gpsimd rings (measured 332 GB/s vs 233 contiguous;
    HBM loads run 424 GB/s but stores only ~233-332, and loads/stores
    share the SDMA engine pool almost additively)
  - UNROLL=20 passes per For_i body: the all-engine back-edge cost
    amortizes, but bodies beyond ~25 passes regress (instruction-stream
    pressure), so 16-20 is the sweet spot
"""

import os
import sys

if "/opt/trn_rl_repo" not in sys.path:
    sys.path.insert(0, "/opt/trn_rl_repo")

import ml_dtypes
import numpy as np

import concourse.bacc as bacc
import concourse.tile as tile
from concourse import mybir
from concourse.bass_interp import get_hw_module
from concourse.bass_utils import run_bass_kernel_spmd

N, K, D = 65536, 64, 128
NCORES = 8
NS = N // NCORES  # 8192 rows per core
NSUPER = int(os.environ.get("V4_NSUPER", "4"))
SUP_N = NS // NSUPER
TPS = SUP_N // 128  # tiles (of 128 rows) per superstep
TPB = 8
NB = 2 * TPB + 2
NBANKS = NS // (TPB * 128)  # 8
SBANKS = TPS // TPB  # psum banks per superstep

WARMUP_MM = int(os.environ.get("V4_WARMUP_MM", "10"))
UNROLL = int(os.environ.get("V4_UNROLL", "20"))
POOL_GROUPS = int(os.environ.get("V4_POOL_GROUPS", "16"))  # of TPS=16
Z_CHUNKS = int(os.environ.get("V4_Z_CHUNKS", "1"))
Q_CHUNKS = int(os.environ.get("V4_Q_CHUNKS", "2"))
SRECIP = os.environ.get("V4_SRECIP", "act")  # act | dve
NOHALVE = int(os.environ.get("V4_NOHALVE", "1"))
# queue for each q-store chunk, cycled: sync(SP) / scalar(ACT) / gpsimd(Pool)
QDMA = os.environ.get("V4_QDMA", "sync,gpsimd").split(",")
# timing-only ablations (break correctness; used by timeonly.py)
NO_ZLOAD = int(os.environ.get("V4_NO_ZLOAD", "0"))
NO_QSTORE = int(os.environ.get("V4_NO_QSTORE", "0"))

FP8 = mybir.dt.float8e4
FP8_NP = mybir.dt.np(FP8)

_CACHE = {}


def _act_recip(nc, out, in_, accum_out=None):
    """Emit InstActivation(Reciprocal) directly; bass's activation() refuses
    it for softmax-grade accuracy reasons that don't apply at our 2e-2
    tolerance (HW-measured: 1.2e-5 max rel err over [0.05, 500])."""
    eng = nc.scalar
    inputs = [eng.lower_ap(in_)]
    for arg in (0.0, 1.0, 0.0):  # bias, scale, alpha
        inputs.append(mybir.ImmediateValue(dtype=mybir.dt.float32, value=arg))
    outputs = [eng.lower_ap(out)]
    if accum_out is not None:
        outputs.append(eng.lower_ap(accum_out))
    return eng.add_instruction(
        mybir.InstActivation(
            name=nc.get_next_instruction_name(),
            func=mybir.ActivationFunctionType.Reciprocal,
            ins=inputs,
            outs=outputs,
        )
    )


def _build_nc(iters=1):
    f32 = mybir.dt.float32
    bf16 = mybir.dt.bfloat16
    nc = bacc.Bacc(
        "TRN2",
        target_bir_lowering=False,
        debug=False,
        enable_asserts=False,
        num_devices=NCORES,
    )
    zT = nc.dram_tensor("zT", [D, NS], FP8, kind="ExternalInput").ap()
    cTm2 = nc.dram_tensor("cTm2", [D, K], bf16, kind="ExternalInput").ap()
    blhs = nc.dram_tensor("blhs", [NB, NBANKS * 128], bf16, kind="ExternalInput").ap()
    brhs = nc.dram_tensor("brhs", [NB, TPB * K], bf16, kind="ExternalInput").ap()
    q = nc.dram_tensor("q", [NS, K], bf16, kind="ExternalOutput").ap()

    # row n = s*2048 + p*16 + u; per-pass store view: [p, s, (u k)]
    q_pass = q.rearrange("(s p u) k -> p s (u k)", p=128, u=TPS)

    with tile.TileContext(nc) as tc:
        with (
            tc.tile_pool(name="const", bufs=1) as const_pool,
            tc.tile_pool(name="zin", bufs=3) as zin_pool,
            tc.tile_pool(name="num", bufs=6) as num_pool,
            tc.tile_pool(name="half", bufs=6) as half_pool,
            tc.tile_pool(name="qst", bufs=2) as qst_pool,
            tc.tile_pool(name="small", bufs=16) as small_pool,
            tc.tile_pool(
                name="psum", bufs=min(4, 8 // SBANKS), space="PSUM"
            ) as psum_pool,
        ):
            # zeros for PE warmup: memset early on DVE (idle at t=0)
            warm = const_pool.tile([128, 128], bf16)
            if WARMUP_MM:
                nc.vector.memset(warm[:], 0.0)

            # consts spread across queues so they land in parallel
            c_sb = const_pool.tile([D, K], bf16)
            nc.scalar.dma_start(c_sb[:], cTm2[:])
            blhs_sb = const_pool.tile([NB, NBANKS * 128], bf16)
            nc.gpsimd.dma_start(blhs_sb[:], blhs[:])
            brhs_sb = const_pool.tile([NB, TPB * K], bf16)
            nc.scalar.dma_start(brhs_sb[:], brhs[:])

            # dummy recip so the ACT Reciprocal table-set load (~2.7us)
            # happens at program start, outside the steady-state loop
            dumm = const_pool.tile([128, 1], f32)
            nc.vector.memset(dumm[:], 1.0)
            dummo = const_pool.tile([128, 1], f32)
            _act_recip(nc, dummo[:], dumm[:])

            def z_load(first=False):
                zt = zin_pool.tile([D, NS], FP8, tag="zt")
                if NO_ZLOAD:
                    nc.vector.memset(zt[:, :4], 0.0)
                    return zt
                nchunk = 4 if first else Z_CHUNKS
                step = NS // nchunk
                for c in range(nchunk):
                    nc.sync.dma_start(
                        zt[:, c * step : (c + 1) * step],
                        zT[:, c * step : (c + 1) * step],
                    )
                return zt

            def super_head(s, zt, first=False):
                """PE matmuls for super s, then the big ACT reciprocal.
                Emitted ahead of the previous super's tail so ACT's FIFO
                stream never stalls waiting for the DVE reduce."""
                ps = psum_pool.tile([128, TPS * K], f32, tag="ps")
                if first and WARMUP_MM:
                    for w in range(WARMUP_MM):
                        nc.tensor.matmul(
                            ps[:, :128],
                            warm[:],
                            warm[:],
                            start=True,
                            stop=True,
                            skip_group_check=True,
                        )
                for h in range(SBANKS):
                    b = SBANKS * s + h
                    for t in range(TPB):
                        u = h * TPB + t
                        nc.tensor.matmul(
                            ps[:, u * K : (u + 1) * K],
                            zt[:, (s * TPS + u) * 128 : (s * TPS + u + 1) * 128],
                            c_sb[:],
                            start=(t == 0),
                            stop=False,
                        )
                    nc.tensor.matmul(
                        ps[:, h * TPB * K : (h + 1) * TPB * K],
                        blhs_sb[:, b * 128 : (b + 1) * 128],
                        brhs_sb[:],
                        start=False,
                        stop=True,
                    )

                # big reciprocal on ACT: PSUM f32 -> SBUF bf16
                num = num_pool.tile([128, TPS * K], bf16, tag="num")
                _act_recip(nc, num[:], ps[:])
                return num

            def super_tail(s, num, qstage):
                srow = small_pool.tile([128, TPS], f32, tag="s")
                if NOHALVE:
                    nc.vector.reduce_sum(
                        out=srow[:],
                        in_=num[:].rearrange("p (u k) -> p u k", k=K),
                        axis=mybir.AxisListType.X,
                    )
                else:
                    # halving add then 32-wide segmented reduce, both on DVE
                    nh = half_pool.tile([128, TPS * K // 2], bf16, tag="nh")
                    nv = num[:].rearrange("p (u k) -> p u k", k=K)
                    nc.vector.tensor_add(
                        nh[:].rearrange("p (u k) -> p u k", k=K // 2),
                        nv[:, :, : K // 2],
                        nv[:, :, K // 2 :],
                    )
                    nc.vector.reduce_sum(
                        out=srow[:],
                        in_=nh[:].rearrange("p (u k) -> p u k", k=K // 2),
                        axis=mybir.AxisListType.X,
                    )
                sinv = small_pool.tile([128, TPS], f32, tag="sinv")
                if SRECIP == "act":
                    _act_recip(nc, sinv[:], srow[:])
                else:
                    nc.vector.reciprocal(out=sinv[:], in_=srow[:])

                # normalize: q = num * sinv, split Pool / DVE
                qs = qstage[:, s * TPS * K : (s + 1) * TPS * K]
                g = POOL_GROUPS
                if g > 0:
                    nc.gpsimd.tensor_mul(
                        qs[:, : g * K].rearrange("p (u k) -> p u k", k=K),
                        num[:, : g * K].rearrange("p (u k) -> p u k", k=K),
                        sinv[:, :g].broadcast_to([128, g, K]),
                    )
                if g < TPS:
                    nc.vector.tensor_mul(
                        qs[:, g * K :].rearrange("p (u k) -> p u k", k=K),
                        num[:, g * K :].rearrange("p (u k) -> p u k", k=K),
                        sinv[:, g:].broadcast_to([128, TPS - g, K]),
                    )

            def body(first=False):
                zt = z_load(first=first)
                qstage = qst_pool.tile([128, NSUPER * TPS * K], bf16, tag="qst")
                nums = {}
                for s in range(NSUPER):
                    nums[s] = super_head(s, zt, first=(first and s == 0))
                    if s >= 1:
                        super_tail(s - 1, nums[s - 1], qstage)
                super_tail(NSUPER - 1, nums[NSUPER - 1], qstage)
                if NO_QSTORE:
                    return
                qsv = qstage[:].rearrange("p (s x) -> p s x", x=TPS * K)
                step = NSUPER // Q_CHUNKS
                for c in range(Q_CHUNKS):
                    eng = getattr(nc, QDMA[c % len(QDMA)])
                    eng.dma_start(
                        q_pass[:, c * step : (c + 1) * step, :],
                        qsv[:, c * step : (c + 1) * step, :],
                    )

            if iters == 1:
                body(first=True)
            else:
                if WARMUP_MM:
                    ps_warm = psum_pool.tile([128, TPS * K], f32, tag="ps")
                    for w in range(WARMUP_MM):
                        nc.tensor.matmul(
                            ps_warm[:, :128],
                            warm[:],
                            warm[:],
                            start=True,
                            stop=True,
                            skip_group_check=True,
                        )
                u = UNROLL if iters % UNROLL == 0 and iters > UNROLL else 1
                with tc.For_i(0, iters // u, 1, staggered_reset=True):
                    for _ in range(u):
                        body(first=False)

    nc.compile()
    nc.m = get_hw_module(nc.m)
    return nc


def _get_nc():
    if "nc" not in _CACHE:
        _CACHE["nc"] = _build_nc()
    return _CACHE["nc"]


def _hilo(x):
    hi = x.astype(ml_dtypes.bfloat16)
    lo = (x - hi.astype(np.float64)).astype(ml_dtypes.bfloat16)
    return hi, lo


def _host_prep(z, centroids):
    z = np.asarray(z, dtype=np.float32)
    c = np.asarray(centroids, dtype=np.float32)

    cm2_bf = ((-2.0 * c.T).astype(ml_dtypes.bfloat16)).astype(np.float64)  # [D,K]
    c_eff = -0.5 * cm2_bf
    csq1 = 1.0 + (c_eff**2).sum(axis=0)
    csq1_hi, csq1_lo = _hilo(csq1)

    brhs = np.zeros((NB, TPB * K), dtype=ml_dtypes.bfloat16)
    for t in range(TPB):
        brhs[t, t * K : (t + 1) * K] = 1.0
        brhs[TPB + t, t * K : (t + 1) * K] = 1.0
    brhs[2 * TPB, :] = np.tile(csq1_hi, TPB)
    brhs[2 * TPB + 1, :] = np.tile(csq1_lo, TPB)

    in_maps = []
    for i in range(NCORES):
        zs = z[i * NS : (i + 1) * NS]
        z_perm = (
            zs.reshape(NSUPER, 128, TPS, D).transpose(0, 2, 1, 3).reshape(NS, D)
        )
        zT8 = np.ascontiguousarray(z_perm.T).astype(FP8_NP)

        z_eff = zT8.astype(np.float64).T
        zsq_perm = (z_eff**2).sum(axis=1)
        zsq_hi, zsq_lo = _hilo(zsq_perm)
        blhs = np.empty((NB, NBANKS * 128), dtype=ml_dtypes.bfloat16)
        hi = zsq_hi.reshape(NSUPER, TPS, 128).reshape(NSUPER, 2, TPB, 128)
        lo = zsq_lo.reshape(NSUPER, TPS, 128).reshape(NSUPER, 2, TPB, 128)
        blhs[:TPB] = hi.transpose(2, 0, 1, 3).reshape(TPB, -1)
        blhs[TPB : 2 * TPB] = lo.transpose(2, 0, 1, 3).reshape(TPB, -1)
        blhs[2 * TPB :] = 1.0
        in_maps.append(
            {
                "zT": zT8,
                "cTm2": cm2_bf.astype(ml_dtypes.bfloat16),
                "blhs": blhs,
                "brhs": brhs,
            }
        )
    return in_maps


def kernel(z, centroids):
    nc = _get_nc()
    in_maps = _host_prep(z, centroids)
    res = run_bass_kernel_spmd(nc, in_maps, list(range(NCORES)))
    out = np.concatenate(
        [np.asarray(res.results[i]["q"]) for i in range(NCORES)], axis=0
    )
    return out.astype(np.float32)
